# revision 36
# baseline (speedup 1.0000x reference)
"""Trainium2 Bass kernel for MembraneNet (PINN forward + analytic PDE residual).

Math: per collocation point p=(x,y):
  u(p)  = Wout . h3 + bout           (4-layer tanh MLP, H=64)
  PDE   = K*(uxx+uyy) + Kx*ux + Ky*uy + f
Forward-mode propagation of (h, gx, gy, lap) per layer:
  z  = W h + b ;  h' = tanh(z) ;  d = 1-h'^2 ;  s = -2 h' d
  zx = W gx    ;  gx' = d . zx      (stored as dbar.zx = -d.zx -> sign
  zy = W gy    ;  gy' = d . zy       alternates per layer; after 4 layers
  zl = W lap   ;  lap' = s.(zx^2+zy^2) + d.zl      it is back to true sign)

The laplacian path is carried as two HALF/NEGATED components so every DVE op
is a plain tensor_tensor (scalar_tensor_tensor has no 2x mode):
  dd' = (dbar.q).h = s.q/2          ee' = dbar.zl = -d.zl
  lap = 2*dd' - ee'  -> folded into the matmuls via pre-scaled weight tiles
  (2*W^T for the dd' pass, -W^T for the ee' pass, and 2*Wout / -Wout columns
  in the output-reduction lhsT).

zx|zy live in ONE [128, 2F] PSUM tile: a single Act square covers both.
zl crosses PSUM->SBUF via an Act copy (so ee' runs 2x on DVE) except in the
last layer, where the copy's queueing latency is on the critical path and a
1x PSUM-operand tensor_tensor is cheaper end-to-end. The output reduction is
split into two PSUM tiles so the ux/uy/u reshape + assembly overlap the
laplacian tail.

Layout: batch 16384 over 8 cores (2048/core); 2 chunks of 1024 points on
SBUF partitions 0-63 / 64-127, block-diagonal weights, fp16 throughout
(PE 16-bit streaming; DVE 2x tensor_tensor, 4x tensor_scalar).
"""

import sys

sys.path.insert(0, "/opt/trn_rl_repo")

import numpy as np
from contextlib import ExitStack

import concourse.bass as bass
import concourse.mybir as mybir
import concourse.tile as tile
from concourse.masks import make_identity

B = 16384
H = 64
L = 4
NCORES = 8
BC = B // NCORES          # 2048 points per core
F = BC // 2               # 1024 free-dim columns (2 chunks stacked)
NH = 512                  # matmul free-dim per instruction (1 PSUM bank fp32)
NMM = F // NH
FT = BC // 128            # 16: free dim of final per-point [128, FT] tiles

f32 = mybir.dt.float32
f16 = mybir.dt.float16
AF = mybir.ActivationFunctionType
OP = mybir.AluOpType

MM_DT = f16               # matmul operand dtype
EW_DT = f16               # elementwise dtype

WB_BIAS = 3 * H * H       # offset of the 8x128 bias block in wb


def _legalize_sync_waits(bj: bytes) -> bytes:
    """The walrus in this container accepts at most ONE on_wait per
    instruction, but Tile emits several. Move excess waits into standalone
    EventSemaphore instructions right before the owner (same engine, so the
    sequencer executes them first)."""
    import json

    m = json.loads(bj)
    n = 0
    for fn in m.get("functions", []):
        for blk in fn.get("blocks", []):
            out = []
            for ins in blk.get("instructions", []):
                si = ins.get("sync_info") or {}
                waits = si.get("on_wait") or []
                if len(waits) > 1:
                    for w in waits[:-1]:
                        n += 1
                        out.append(
                            {
                                "name": f"lsw_{n}",
                                "opcode": "EventSemaphore",
                                "engine": ins["engine"],
                                "ins": [],
                                "outs": [],
                                "debug": ins.get("debug", 0),
                                "sync_info": {"on_update": [], "on_wait": [w]},
                            }
                        )
                    si["on_wait"] = waits[-1:]
                out.append(ins)
            blk["instructions"] = out
    return json.dumps(m).encode()


def build_nc(mm_dt=MM_DT, ew_dt=EW_DT):
    nc = bass.Bass()

    # ---- I/O (host-packed; see make_in_maps) ----
    xc_d = nc.dram_tensor("xc", [BC], f16, kind="ExternalInput")
    yc_d = nc.dram_tensor("yc", [BC], f16, kind="ExternalInput")
    w0r_d = nc.dram_tensor("w0r", [2 * H], f16, kind="ExternalInput")
    wb_d = nc.dram_tensor("wb", [WB_BIAS + 8 * 128], f32, kind="ExternalInput")
    kkf_d = nc.dram_tensor("kkf", [4 * BC], f32, kind="ExternalInput")
    u_d = nc.dram_tensor("u", [BC], f32, kind="ExternalOutput")
    pde_d = nc.dram_tensor("pde", [BC], f32, kind="ExternalOutput")

    with tile.TileContext(nc) as tc, ExitStack() as ctx:
        const = ctx.enter_context(tc.tile_pool(name="const", bufs=1))
        sb = ctx.enter_context(tc.tile_pool(name="sb", bufs=2))
        ps = ctx.enter_context(tc.tile_pool(name="ps", bufs=1, space="PSUM"))

        # ---- vector: xyT memset first (it gates the coordinate DMAs) ----
        xyT = const.tile([128, F], mm_dt, tag="xyT")
        nc.vector.memset(xyT[:], 0.0)
        scr = const.tile([128, NH], mm_dt, tag="scr")
        nc.vector.memset(scr[:], 0.0)
        wt0 = const.tile([128, 128], mm_dt, tag="wt0")
        nc.vector.memset(wt0[:], 0.0)
        ztiny = const.tile([1, 2], f32, tag="ztiny")
        nc.vector.memset(ztiny[:], 0.0)
        wlAf = const.tile([128, 24], f32, tag="wlAf")
        nc.vector.memset(wlAf[:], 0.0)
        wlBf = const.tile([128, 16], f32, tag="wlBf")
        nc.vector.memset(wlBf[:], 0.0)
        wnat = []
        for k in range(1, L):
            w = const.tile([128, 128], f32, tag=f"wnat{k}")
            nc.vector.memset(w[0:H, H:128], 0.0)
            nc.vector.memset(w[H:128, 0:H], 0.0)
            wnat.append(w)

        # ---- scalar queue: dummy tanh first (ACT table load overlaps
        # the preamble DMAs) ----
        dum = const.tile([1, 2], ew_dt, tag="dum")
        nc.scalar.activation(dum[:], ztiny[:], AF.Tanh)

        # ---- sync queue: all input DMAs, most-urgent first ----
        nc.sync.dma_start(out=xyT[0:1, :], in_=xc_d[0:F][None, :])
        nc.sync.dma_start(out=xyT[1:2, :], in_=yc_d[0:F][None, :])
        nc.sync.dma_start(out=xyT[H : H + 1, :], in_=xc_d[F:BC][None, :])
        nc.sync.dma_start(out=xyT[H + 1 : H + 2, :], in_=yc_d[F:BC][None, :])
        for i, k in enumerate(range(1, L)):
            wk = wb_d[(k - 1) * H * H : k * H * H].rearrange(
                "(p c) -> p c", p=H
            )
            nc.sync.dma_start(out=wnat[i][0:H, 0:H], in_=wk)
            nc.sync.dma_start(out=wnat[i][H:128, H:128], in_=wk)

        # ---- gpsimd queue: L0 weights, bias block, identity, kkf ----
        w0v = w0r_d[:].rearrange("(p c) -> p c", p=2)
        nc.gpsimd.dma_start(out=wt0[0:2, 0:H], in_=w0v)
        nc.gpsimd.dma_start(out=wt0[H : H + 2, H:128], in_=w0v)
        stg2 = const.tile([128, 128], f32, tag="stg2")
        nc.gpsimd.dma_start(
            out=stg2[0:8, :],
            in_=wb_d[WB_BIAS : WB_BIAS + 1024].rearrange("(p c) -> p c", p=8),
        )
        ident = const.tile([128, 128], f32, tag="ident")
        make_identity(nc, ident[:])
        kkft = const.tile([128, 4 * FT], f32, tag="kkft")
        nc.gpsimd.dma_start(
            out=kkft[:].rearrange("p (q k) -> p q k", q=4),
            in_=kkf_d[:].rearrange("(q p k) -> p q k", q=4, p=128, k=FT),
        )

        # ---- HAM warm-up: ~4.5us of back-to-back dummy matmuls while the
        # preamble DMAs land, so the PE clocks up to 2.4 GHz before the
        # real matmuls start (idle PE stays throttled at 1.2 GHz) ----
        wup = ps.tile([128, NH], f32, tag="zp")
        for _ in range(9):
            nc.tensor.matmul(
                wup[:, :], scr[:, 0:128], scr[:, :], start=True, stop=True
            )

        # ---- bias/Wout/W0 columns via one transpose: bwc col j =
        # [b0..b3, Wout, W0x, W0y, bout] ----
        bw_ps = ps.tile([128, 128], f32, tag="zxp")
        nc.tensor.transpose(bw_ps[:], stg2[:], ident[:])
        bwc = const.tile([128, 8], f32, tag="bwc")
        nc.vector.tensor_copy(bwc[:], bw_ps[:, 0:8])
        bcol = [bwc[:, k : k + 1] for k in range(L)]

        # ---- weight transposes -> fp16 block-diag lhsT (+ scaled variants
        # 2*W^T / -W^T for the laplacian-source passes) ----
        WT, WTd, WTe = [None], [None, None], [None, None]
        pstags = ["zlp", "zyp", "zlp"]
        for i, k in enumerate(range(1, L)):
            wt_ps = ps.tile([128, 128], f32, tag=pstags[i])
            nc.tensor.transpose(wt_ps[:], wnat[i][:], ident[:])
            wt = const.tile([128, 128], mm_dt, tag=f"wt{k}")
            nc.vector.tensor_copy(wt[:], wt_ps[:, :])
            WT.append(wt)
            if k >= 2:
                wtd = const.tile([128, 128], mm_dt, tag=f"wtd{k}")
                wte = const.tile([128, 128], mm_dt, tag=f"wte{k}")
                nc.vector.tensor_scalar_mul(wtd[:], wt[:], 2.0)
                nc.vector.tensor_scalar_mul(wte[:], wt[:], -1.0)
                WTd.append(wtd)
                WTe.append(wte)

        # ---- output-reduction lhsTs.
        # wlA: 3 groups of 8 -> rp1 rows (r, r+4): g0=ux(r0), g1=uy(r1),
        #      g2=u(r3).
        # wlB: 2 groups of 8 -> rp2 rows (0, 4): g0=2Wout (dd'), g1=-Wout
        #      (ee') accumulating S = 2dd' - ee'. ----
        w2c = const.tile([128, 1], f32, tag="w2c")
        wnc = const.tile([128, 1], f32, tag="wnc")
        nc.vector.tensor_scalar_mul(w2c[:], bwc[:, 4:5], 2.0)
        nc.vector.tensor_scalar_mul(wnc[:], bwc[:, 4:5], -1.0)
        for g, row in ((0, 0), (1, 1), (2, 3)):
            c = 8 * g + row
            nc.vector.tensor_copy(wlAf[0:H, c : c + 1], bwc[0:H, 4:5])
            nc.vector.tensor_copy(wlAf[H:128, c + 4 : c + 5], bwc[H:128, 4:5])
        for g, src in ((0, w2c[:]), (1, wnc[:])):
            c = 8 * g
            nc.vector.tensor_copy(wlBf[0:H, c : c + 1], src[0:H])
            nc.vector.tensor_copy(wlBf[H:128, c + 4 : c + 5], src[H:128])
        wlA = const.tile([128, 24], mm_dt, tag="wlA")
        wlB = const.tile([128, 16], mm_dt, tag="wlB")
        nc.vector.tensor_copy(wlA[:], wlAf[:])
        nc.vector.tensor_copy(wlB[:], wlBf[:])

        # q0p2 = 2*(W0x^2 + W0y^2) per partition (layer-0 laplacian source)
        w5sq = const.tile([128, 1], f32, tag="w5sq")
        w6sq = const.tile([128, 1], f32, tag="w6sq")
        q0s = const.tile([128, 1], f32, tag="q0s")
        q0p2 = const.tile([128, 1], f32, tag="q0p2")
        nc.vector.tensor_mul(w5sq[:], bwc[:, 5:6], bwc[:, 5:6])
        nc.vector.tensor_mul(w6sq[:], bwc[:, 6:7], bwc[:, 6:7])
        nc.vector.tensor_add(q0s[:], w5sq[:], w6sq[:])
        nc.vector.tensor_scalar_mul(q0p2[:], q0s[:], 2.0)

        def mm_pass(out_ps, lhsT, rhs, start=True, stop=True):
            for j in range(NMM):
                s = slice(j * NH, (j + 1) * NH)
                nc.tensor.matmul(
                    out_ps[:, s], lhsT[:], rhs[:, s], start=start, stop=stop
                )

        # ---- layer 0: only tanh on Act; the rest on DVE so layer 1's
        # matmul inputs appear as early as possible ----
        zp = ps.tile([128, F], f32, tag="zp")
        mm_pass(zp, wt0, xyT)
        h = sb.tile([128, F], mm_dt, tag="h")
        nc.scalar.activation(h[:], zp[:], AF.Tanh, bias=bcol[0], scale=1.0)
        hsq = sb.tile([128, F], ew_dt, tag="hsq")
        dbar = sb.tile([128, F], ew_dt, tag="dbar")
        gxs = sb.tile([128, F], mm_dt, tag="gxs")
        gys = sb.tile([128, F], mm_dt, tag="gys")
        m2 = sb.tile([128, F], ew_dt, tag="m2")
        lap0 = sb.tile([128, F], mm_dt, tag="dd")
        nc.vector.tensor_mul(hsq[:], h[:], h[:])
        nc.vector.tensor_scalar_add(dbar[:], hsq[:], -1.0)
        nc.vector.tensor_scalar_mul(gxs[:], dbar[:], bwc[:, 5:6])
        nc.vector.tensor_scalar_mul(gys[:], dbar[:], bwc[:, 6:7])
        nc.vector.tensor_mul(m2[:], h[:], dbar[:])            # = s/2
        nc.vector.tensor_scalar_mul(lap0[:], m2[:], q0p2[:])  # = s*q0
        lsrc = [(WT[1], lap0)]

        # ---- layers 1..3 ----
        for k in range(1, L):
            last = k == L - 1
            zp = ps.tile([128, F], f32, tag="zp")
            zxp = ps.tile([128, F], f32, tag="zxp")
            zyp = ps.tile([128, F], f32, tag="zyp")
            zlp = ps.tile([128, F], f32, tag="zlp")
            mm_pass(zp, WT[k], h)
            mm_pass(zxp, WT[k], gxs)
            mm_pass(zyp, WT[k], gys)
            for i, (lh, ls) in enumerate(lsrc):
                mm_pass(
                    zlp, lh, ls, start=(i == 0), stop=(i == len(lsrc) - 1)
                )

            h = sb.tile([128, F], mm_dt, tag="h")
            hsq = sb.tile([128, F], ew_dt, tag="hsq")
            zxsq = sb.tile([128, F], ew_dt, tag="zxsq")
            zysq = sb.tile([128, F], ew_dt, tag="zysq")
            nc.scalar.activation(h[:], zp[:], AF.Tanh, bias=bcol[k], scale=1.0)
            nc.scalar.activation(hsq[:], h[:], AF.Square)
            nc.scalar.activation(zxsq[:], zxp[:], AF.Square)
            nc.scalar.activation(zysq[:], zyp[:], AF.Square)
            if not last:
                zlc = sb.tile([128, F], ew_dt, tag="zlc")
                nc.scalar.copy(zlc[:], zlp[:])

            dbar = sb.tile([128, F], ew_dt, tag="dbar")
            gxs = sb.tile([128, F], mm_dt, tag="gxs")
            gys = sb.tile([128, F], mm_dt, tag="gys")
            q = sb.tile([128, F], ew_dt, tag="q")
            t = sb.tile([128, F], ew_dt, tag="m2")
            dd = sb.tile([128, F], mm_dt, tag="dd")
            ee = sb.tile([128, F], mm_dt, tag="ee")
            nc.vector.tensor_scalar_add(dbar[:], hsq[:], -1.0)
            if last:
                # S-critical chain first (PSUM-direct ee skips the Act
                # copy); gx/gy only feed the ux/uy reductions and can wait
                nc.vector.tensor_mul(ee[:], dbar[:], zlp[:])   # = -d.zl
                nc.vector.tensor_add(q[:], zxsq[:], zysq[:])
                nc.vector.tensor_mul(t[:], dbar[:], q[:])
                nc.vector.tensor_mul(dd[:], t[:], h[:])        # = s.q/2
                nc.vector.tensor_mul(gxs[:], dbar[:], zxp[:])
                nc.vector.tensor_mul(gys[:], dbar[:], zyp[:])
            else:
                nc.vector.tensor_mul(gxs[:], dbar[:], zxp[:])
                nc.vector.tensor_mul(gys[:], dbar[:], zyp[:])
                nc.vector.tensor_add(q[:], zxsq[:], zysq[:])
                nc.vector.tensor_mul(t[:], dbar[:], q[:])
                nc.vector.tensor_mul(dd[:], t[:], h[:])        # = s.q/2
                nc.vector.tensor_mul(ee[:], dbar[:], zlc[:])   # = -d.zl
                lsrc = [(WTd[k + 1], dd), (WTe[k + 1], ee)]

        # ---- output reductions, split so the ux/uy/u tail overlaps the
        # laplacian tail. rp1 rows: 0=ux_A 1=uy_A 3=u_A (+4 chunk B);
        # rp2 rows: 0=S_A 4=S_B ----
        rp2 = ps.tile([8, F], f32, tag="zlp")
        srcs2 = [(1, ee[:]), (0, dd[:])]
        for i, (g, src) in enumerate(srcs2):
            for j in range(NMM):
                s = slice(j * NH, (j + 1) * NH)
                nc.tensor.matmul(
                    rp2[:, s], wlB[:, 8 * g : 8 * g + 8], src[:, s],
                    start=(i == 0), stop=(i == len(srcs2) - 1),
                )
        rp1 = ps.tile([8, F], f32, tag="zp")
        srcs1 = [(2, h[:]), (0, gxs[:]), (1, gys[:])]
        for i, (g, src) in enumerate(srcs1):
            for j in range(NMM):
                s = slice(j * NH, (j + 1) * NH)
                nc.tensor.matmul(
                    rp1[:, s], wlA[:, 8 * g : 8 * g + 8], src[:, s],
                    start=(i == 0), stop=(i == len(srcs1) - 1),
                )

        # ---- reshape rows -> per-point [128, 4*FT] tile.
        # T cols: ux 0:16 | uy 16:32 | S 32:48 | u 48:64 ----
        red2 = sb.tile([8, F], f32, tag="red2")
        nc.scalar.copy(red2[:], rp2[:])
        T = sb.tile([128, 4 * FT], f32, tag="T")
        for hh, eng in ((0, nc.sync), (1, nc.gpsimd)):
            eng.dma_start(
                out=T[hh * H : (hh + 1) * H, 2 * FT : 3 * FT],
                in_=red2[4 * hh : 4 * hh + 1, :],
            )
        red1 = sb.tile([8, F], f32, tag="red1")
        nc.scalar.copy(red1[:], rp1[:])
        engs = [nc.sync, nc.scalar, nc.gpsimd]
        for i, (hh, q_) in enumerate(
            (hh, q_) for q_ in (0, 1, 3) for hh in range(2)
        ):
            engs[i % 3].dma_start(
                out=T[hh * H : (hh + 1) * H, q_ * FT : (q_ + 1) * FT],
                in_=red1[4 * hh + q_ : 4 * hh + q_ + 1, :],
            )

        # ---- final assembly (S part first — it lands first now) ----
        m1 = sb.tile([128, 2 * FT], f32, tag="m1")
        p1 = sb.tile([128, FT], f32, tag="p1")
        u_fin = sb.tile([128, FT], f32, tag="u_fin")
        m2f = sb.tile([128, FT], f32, tag="m2f")
        p2 = sb.tile([128, FT], f32, tag="p2")
        pde = sb.tile([128, FT], f32, tag="pde")
        nc.vector.tensor_mul(
            m2f[:], T[:, 2 * FT : 3 * FT], kkft[:, 2 * FT : 3 * FT]
        )
        nc.vector.tensor_add(p2[:], m2f[:], kkft[:, 3 * FT : 4 * FT])
        nc.vector.tensor_mul(m1[:], T[:, 0 : 2 * FT], kkft[:, 0 : 2 * FT])
        nc.vector.tensor_add(p1[:], m1[:, 0:FT], m1[:, FT : 2 * FT])
        nc.vector.tensor_add(pde[:], p1[:], p2[:])
        nc.sync.dma_start(
            out=pde_d[:].rearrange("(p j) -> p j", p=128), in_=pde[:]
        )
        nc.vector.tensor_scalar_add(
            u_fin[:], T[:, 3 * FT : 4 * FT], bwc[:, 7:8]
        )
        nc.scalar.dma_start(
            out=u_d[:].rearrange("(p j) -> p j", p=128), in_=u_fin[:]
        )

    if not nc.is_finalized():
        nc.finalize()
    legalized = _legalize_sync_waits(nc.to_json_bytes())
    nc.to_json_bytes = lambda: legalized
    return nc


_NC = None


def _get_nc():
    global _NC
    if _NC is None:
        # NOTE: walrus LDW-opt stays OFF — it rejects fp16 LDWEIGHTS.
        _NC = build_nc()
    return _NC


def make_in_maps(inputs):
    """Shard full inputs into per-core input maps (host-side packing)."""
    full = {k: np.asarray(v, dtype=np.float32) for k, v in inputs.items()}
    W0 = full["W0"]
    w0r = np.concatenate([W0[:, 0], W0[:, 1]]).astype(np.float16)
    bias_block = np.zeros((8, 128), np.float32)
    for i in range(L):
        bias_block[i] = np.tile(full[f"b{i}"], 2)
    bias_block[4] = np.tile(full["Wout"], 2)
    bias_block[5] = np.tile(W0[:, 0], 2)
    bias_block[6] = np.tile(W0[:, 1], 2)
    bias_block[7] = full["bout"]
    wb = np.concatenate(
        [
            full["W1"].ravel(),
            full["W2"].ravel(),
            full["W3"].ravel(),
            bias_block.ravel(),
        ]
    ).astype(np.float32)

    in_maps = []
    for c in range(NCORES):
        s = slice(c * BC, (c + 1) * BC)
        xy = full["xy"][s]
        kkf = np.concatenate(
            [full["Kx"][s], full["Ky"][s], full["K"][s], full["f"][s]]
        ).astype(np.float32)
        in_maps.append(
            {
                "xc": np.ascontiguousarray(xy[:, 0]).astype(np.float16),
                "yc": np.ascontiguousarray(xy[:, 1]).astype(np.float16),
                "w0r": w0r,
                "wb": wb,
                "kkf": kkf,
            }
        )
    return in_maps


def run(inputs, trace=False, **kw):
    from concourse.bass_utils import run_bass_kernel_spmd

    nc = _get_nc()
    res = run_bass_kernel_spmd(
        nc, make_in_maps(inputs), list(range(NCORES)), trace=trace, **kw
    )
    u = np.concatenate([r["u"] for r in res.results])
    pde = np.concatenate([r["pde"] for r in res.results])
    return (u, pde), res


def kernel(**inputs):
    (u, pde), _ = run(inputs)
    return u, pde


# revision 37
# speedup vs baseline: 1.0943x; 1.0943x over previous
"""Trainium2 Bass kernel for MembraneNet (PINN forward + analytic PDE residual).

Math: per collocation point p=(x,y):
  u(p)  = Wout . h3 + bout           (4-layer tanh MLP, H=64)
  PDE   = K*(uxx+uyy) + Kx*ux + Ky*uy + f
Forward-mode propagation of (h, gx, gy, lap) per layer:
  z  = W h + b ;  h' = tanh(z) ;  d = 1-h'^2 ;  s = -2 h' d
  zx = W gx    ;  gx' = d . zx      (stored as dbar.zx = -d.zx -> sign
  zy = W gy    ;  gy' = d . zy       alternates per layer; after 4 layers
  zl = W lap   ;  lap' = s.(zx^2+zy^2) + d.zl      it is back to true sign)

The laplacian path is carried as two HALF/NEGATED components so every DVE op
is a plain tensor_tensor (scalar_tensor_tensor has no 2x mode):
  dd' = (dbar.q).h = s.q/2          ee' = dbar.zl = -d.zl
  lap = 2*dd' - ee'  -> folded into the matmuls via pre-scaled weight tiles
  (2*W^T for the dd' pass, -W^T for the ee' pass, and 2*Wout / -Wout columns
  in the output-reduction lhsT).

zx|zy live in ONE [128, 2F] PSUM tile: a single Act square covers both.
zl crosses PSUM->SBUF via an Act copy (so ee' runs 2x on DVE) except in the
last layer, where the copy's queueing latency is on the critical path and a
1x PSUM-operand tensor_tensor is cheaper end-to-end. The output reduction is
split into two PSUM tiles so the ux/uy/u reshape + assembly overlap the
laplacian tail.

Layout: batch 16384 over 8 cores (2048/core); 2 chunks of 1024 points on
SBUF partitions 0-63 / 64-127, block-diagonal weights, fp16 throughout
(PE 16-bit streaming; DVE 2x tensor_tensor, 4x tensor_scalar).
"""

import sys

sys.path.insert(0, "/opt/trn_rl_repo")

import numpy as np
from contextlib import ExitStack

import concourse.bass as bass
import concourse.mybir as mybir
import concourse.tile as tile
from concourse.masks import make_identity

B = 16384
H = 64
L = 4
NCORES = 8
BC = B // NCORES          # 2048 points per core
F = BC // 2               # 1024 free-dim columns (2 chunks stacked)
NH = 512                  # matmul free-dim per instruction (1 PSUM bank fp32)
NMM = F // NH
FT = BC // 128            # 16: free dim of final per-point [128, FT] tiles

f32 = mybir.dt.float32
f16 = mybir.dt.float16
AF = mybir.ActivationFunctionType
OP = mybir.AluOpType

MM_DT = f16               # matmul operand dtype
EW_DT = f16               # elementwise dtype

WB_BIAS = 3 * H * H       # offset of the 8x128 bias block in wb


def _legalize_sync_waits(bj: bytes) -> bytes:
    """The walrus in this container accepts at most ONE on_wait per
    instruction, but Tile emits several. Move excess waits into standalone
    EventSemaphore instructions right before the owner (same engine, so the
    sequencer executes them first)."""
    import json

    m = json.loads(bj)
    n = 0
    for fn in m.get("functions", []):
        for blk in fn.get("blocks", []):
            out = []
            for ins in blk.get("instructions", []):
                si = ins.get("sync_info") or {}
                waits = si.get("on_wait") or []
                if len(waits) > 1:
                    for w in waits[:-1]:
                        n += 1
                        out.append(
                            {
                                "name": f"lsw_{n}",
                                "opcode": "EventSemaphore",
                                "engine": ins["engine"],
                                "ins": [],
                                "outs": [],
                                "debug": ins.get("debug", 0),
                                "sync_info": {"on_update": [], "on_wait": [w]},
                            }
                        )
                    si["on_wait"] = waits[-1:]
                out.append(ins)
            blk["instructions"] = out
    return json.dumps(m).encode()


def build_nc(mm_dt=MM_DT, ew_dt=EW_DT):
    nc = bass.Bass()

    # ---- I/O (host-packed; see make_in_maps) ----
    xc_d = nc.dram_tensor("xc", [BC], f16, kind="ExternalInput")
    yc_d = nc.dram_tensor("yc", [BC], f16, kind="ExternalInput")
    w0r_d = nc.dram_tensor("w0r", [2 * H], f16, kind="ExternalInput")
    wb_d = nc.dram_tensor("wb", [WB_BIAS + 8 * 128], f32, kind="ExternalInput")
    kkf_d = nc.dram_tensor("kkf", [4 * BC], f32, kind="ExternalInput")
    u_d = nc.dram_tensor("u", [BC], f32, kind="ExternalOutput")
    pde_d = nc.dram_tensor("pde", [BC], f32, kind="ExternalOutput")

    with tile.TileContext(nc) as tc, ExitStack() as ctx:
        const = ctx.enter_context(tc.tile_pool(name="const", bufs=1))
        sb = ctx.enter_context(tc.tile_pool(name="sb", bufs=2))
        ps = ctx.enter_context(tc.tile_pool(name="ps", bufs=1, space="PSUM"))

        # ---- vector: xyT memset first (it gates the coordinate DMAs) ----
        xyT = const.tile([128, F], mm_dt, tag="xyT")
        nc.vector.memset(xyT[:], 0.0)
        scr = const.tile([128, NH], mm_dt, tag="scr")
        nc.vector.memset(scr[:], 0.0)
        wt0 = const.tile([128, 128], mm_dt, tag="wt0")
        nc.vector.memset(wt0[:], 0.0)
        ztiny = const.tile([1, 2], f32, tag="ztiny")
        nc.vector.memset(ztiny[:], 0.0)
        wlAf = const.tile([128, 24], f32, tag="wlAf")
        nc.vector.memset(wlAf[:], 0.0)
        wlBf = const.tile([128, 16], f32, tag="wlBf")
        nc.vector.memset(wlBf[:], 0.0)
        wnat = []
        for k in range(1, L):
            w = const.tile([128, 128], f32, tag=f"wnat{k}")
            nc.vector.memset(w[0:H, H:128], 0.0)
            nc.vector.memset(w[H:128, 0:H], 0.0)
            wnat.append(w)

        # ---- scalar queue: dummy tanh first (ACT table load overlaps
        # the preamble DMAs) ----
        dum = const.tile([1, 2], ew_dt, tag="dum")
        nc.scalar.activation(dum[:], ztiny[:], AF.Tanh)

        # ---- sync queue: all input DMAs, most-urgent first ----
        nc.sync.dma_start(out=xyT[0:1, :], in_=xc_d[0:F][None, :])
        nc.sync.dma_start(out=xyT[1:2, :], in_=yc_d[0:F][None, :])
        nc.sync.dma_start(out=xyT[H : H + 1, :], in_=xc_d[F:BC][None, :])
        nc.sync.dma_start(out=xyT[H + 1 : H + 2, :], in_=yc_d[F:BC][None, :])
        for i, k in enumerate(range(1, L)):
            wk = wb_d[(k - 1) * H * H : k * H * H].rearrange(
                "(p c) -> p c", p=H
            )
            nc.sync.dma_start(out=wnat[i][0:H, 0:H], in_=wk)
            nc.sync.dma_start(out=wnat[i][H:128, H:128], in_=wk)

        # ---- gpsimd queue: L0 weights, bias block, identity, kkf ----
        w0v = w0r_d[:].rearrange("(p c) -> p c", p=2)
        nc.gpsimd.dma_start(out=wt0[0:2, 0:H], in_=w0v)
        nc.gpsimd.dma_start(out=wt0[H : H + 2, H:128], in_=w0v)
        stg2 = const.tile([128, 128], f32, tag="stg2")
        nc.gpsimd.dma_start(
            out=stg2[0:8, :],
            in_=wb_d[WB_BIAS : WB_BIAS + 1024].rearrange("(p c) -> p c", p=8),
        )
        ident = const.tile([128, 128], f32, tag="ident")
        make_identity(nc, ident[:])
        kkft = const.tile([128, 4 * FT], f32, tag="kkft")
        nc.gpsimd.dma_start(
            out=kkft[:].rearrange("p (q k) -> p q k", q=4),
            in_=kkf_d[:].rearrange("(q p k) -> p q k", q=4, p=128, k=FT),
        )

        # ---- HAM warm-up: ~4.5us of back-to-back dummy matmuls while the
        # preamble DMAs land, so the PE clocks up to 2.4 GHz before the
        # real matmuls start (idle PE stays throttled at 1.2 GHz) ----
        wup = ps.tile([128, NH], f32, tag="zp")
        for _ in range(9):
            nc.tensor.matmul(
                wup[:, :], scr[:, 0:128], scr[:, :], start=True, stop=True
            )

        # ---- bias/Wout/W0 columns via one transpose: bwc col j =
        # [b0..b3, Wout, W0x, W0y, bout] ----
        bw_ps = ps.tile([128, 128], f32, tag="zxyp")
        nc.tensor.transpose(bw_ps[:], stg2[:], ident[:])
        bwc = const.tile([128, 8], f32, tag="bwc")
        nc.vector.tensor_copy(bwc[:], bw_ps[:, 0:8])
        bcol = [bwc[:, k : k + 1] for k in range(L)]

        # ---- weight transposes -> fp16 block-diag lhsT (+ scaled variants
        # 2*W^T / -W^T for the laplacian-source passes) ----
        WT, WTd, WTe = [None], [None, None], [None, None]
        pstags = ["zlp", "zxyp", "zlp"]
        for i, k in enumerate(range(1, L)):
            wt_ps = ps.tile([128, 128], f32, tag=pstags[i])
            nc.tensor.transpose(wt_ps[:], wnat[i][:], ident[:])
            wt = const.tile([128, 128], mm_dt, tag=f"wt{k}")
            nc.vector.tensor_copy(wt[:], wt_ps[:, :])
            WT.append(wt)
            if k >= 2:
                wtd = const.tile([128, 128], mm_dt, tag=f"wtd{k}")
                wte = const.tile([128, 128], mm_dt, tag=f"wte{k}")
                nc.vector.tensor_scalar_mul(wtd[:], wt[:], 2.0)
                nc.vector.tensor_scalar_mul(wte[:], wt[:], -1.0)
                WTd.append(wtd)
                WTe.append(wte)

        # ---- output-reduction lhsTs.
        # wlA: 3 groups of 8 -> rp1 rows (r, r+4): g0=ux(r0), g1=uy(r1),
        #      g2=u(r3).
        # wlB: 2 groups of 8 -> rp2 rows (0, 4): g0=2Wout (dd'), g1=-Wout
        #      (ee') accumulating S = 2dd' - ee'. ----
        w2c = const.tile([128, 1], f32, tag="w2c")
        wnc = const.tile([128, 1], f32, tag="wnc")
        nc.vector.tensor_scalar_mul(w2c[:], bwc[:, 4:5], 2.0)
        nc.vector.tensor_scalar_mul(wnc[:], bwc[:, 4:5], -1.0)
        for g, row in ((0, 0), (1, 1), (2, 3)):
            c = 8 * g + row
            nc.vector.tensor_copy(wlAf[0:H, c : c + 1], bwc[0:H, 4:5])
            nc.vector.tensor_copy(wlAf[H:128, c + 4 : c + 5], bwc[H:128, 4:5])
        for g, src in ((0, w2c[:]), (1, wnc[:])):
            c = 8 * g
            nc.vector.tensor_copy(wlBf[0:H, c : c + 1], src[0:H])
            nc.vector.tensor_copy(wlBf[H:128, c + 4 : c + 5], src[H:128])
        wlA = const.tile([128, 24], mm_dt, tag="wlA")
        wlB = const.tile([128, 16], mm_dt, tag="wlB")
        nc.vector.tensor_copy(wlA[:], wlAf[:])
        nc.vector.tensor_copy(wlB[:], wlBf[:])

        # q0p2 = 2*(W0x^2 + W0y^2) per partition (layer-0 laplacian source)
        w5sq = const.tile([128, 1], f32, tag="w5sq")
        w6sq = const.tile([128, 1], f32, tag="w6sq")
        q0s = const.tile([128, 1], f32, tag="q0s")
        q0p2 = const.tile([128, 1], f32, tag="q0p2")
        nc.vector.tensor_mul(w5sq[:], bwc[:, 5:6], bwc[:, 5:6])
        nc.vector.tensor_mul(w6sq[:], bwc[:, 6:7], bwc[:, 6:7])
        nc.vector.tensor_add(q0s[:], w5sq[:], w6sq[:])
        nc.vector.tensor_scalar_mul(q0p2[:], q0s[:], 2.0)

        def mm_pass(out_ps, lhsT, rhs, start=True, stop=True):
            for j in range(NMM):
                s = slice(j * NH, (j + 1) * NH)
                nc.tensor.matmul(
                    out_ps[:, s], lhsT[:], rhs[:, s], start=start, stop=stop
                )

        # ---- layer 0: only tanh on Act; the rest on DVE so layer 1's
        # matmul inputs appear as early as possible ----
        zp = ps.tile([128, F], f32, tag="zp")
        mm_pass(zp, wt0, xyT)
        h = sb.tile([128, F], mm_dt, tag="h")
        nc.scalar.activation(h[:], zp[:], AF.Tanh, bias=bcol[0], scale=1.0)
        hsq = sb.tile([128, F], ew_dt, tag="hsq")
        dbar = sb.tile([128, F], ew_dt, tag="dbar")
        gxy = sb.tile([128, 2 * F], mm_dt, tag="gxy")
        m2 = sb.tile([128, F], ew_dt, tag="m2")
        lap0 = sb.tile([128, F], mm_dt, tag="dd")
        nc.vector.tensor_mul(hsq[:], h[:], h[:])
        nc.vector.tensor_scalar_add(dbar[:], hsq[:], -1.0)
        nc.vector.tensor_scalar_mul(gxy[:, 0:F], dbar[:], bwc[:, 5:6])
        nc.vector.tensor_scalar_mul(gxy[:, F : 2 * F], dbar[:], bwc[:, 6:7])
        nc.vector.tensor_mul(m2[:], h[:], dbar[:])            # = s/2
        nc.vector.tensor_scalar_mul(lap0[:], m2[:], q0p2[:])  # = s*q0
        lsrc = [(WT[1], lap0)]

        # ---- layers 1..3 ----
        for k in range(1, L):
            last = k == L - 1
            zp = ps.tile([128, F], f32, tag="zp")
            zxyp = ps.tile([128, 2 * F], f32, tag="zxyp")
            zlp = ps.tile([128, F], f32, tag="zlp")
            mm_pass(zp, WT[k], h)
            mm_pass(zxyp[:, 0:F], WT[k], gxy[:, 0:F])
            mm_pass(zxyp[:, F : 2 * F], WT[k], gxy[:, F : 2 * F])
            for i, (lh, ls) in enumerate(lsrc):
                mm_pass(
                    zlp, lh, ls, start=(i == 0), stop=(i == len(lsrc) - 1)
                )

            h = sb.tile([128, F], mm_dt, tag="h")
            hsq = sb.tile([128, F], ew_dt, tag="hsq")
            sq2 = sb.tile([128, 2 * F], ew_dt, tag="sq2")
            nc.scalar.activation(h[:], zp[:], AF.Tanh, bias=bcol[k], scale=1.0)
            nc.scalar.activation(hsq[:], h[:], AF.Square)
            nc.scalar.activation(sq2[:], zxyp[:], AF.Square)
            if not last:
                zlc = sb.tile([128, F], ew_dt, tag="zlc")
                nc.scalar.copy(zlc[:], zlp[:])

            dbar = sb.tile([128, F], ew_dt, tag="dbar")
            gxy = sb.tile([128, 2 * F], mm_dt, tag="gxy")
            q = sb.tile([128, F], ew_dt, tag="q")
            t = sb.tile([128, F], ew_dt, tag="m2")
            dd = sb.tile([128, F], mm_dt, tag="dd")
            ee = sb.tile([128, F], mm_dt, tag="ee")
            nc.vector.tensor_scalar_add(dbar[:], hsq[:], -1.0)
            if last:
                # S-critical chain first (PSUM-direct ee skips the Act
                # copy); gx/gy only feed the ux/uy reductions and can wait
                nc.vector.tensor_mul(ee[:], dbar[:], zlp[:])   # = -d.zl
                nc.vector.tensor_add(q[:], sq2[:, 0:F], sq2[:, F : 2 * F])
                nc.vector.tensor_mul(t[:], dbar[:], q[:])
                nc.vector.tensor_mul(dd[:], t[:], h[:])        # = s.q/2
                nc.vector.tensor_mul(gxy[:, 0:F], dbar[:], zxyp[:, 0:F])
                nc.vector.tensor_mul(
                    gxy[:, F : 2 * F], dbar[:], zxyp[:, F : 2 * F]
                )
            else:
                nc.vector.tensor_mul(gxy[:, 0:F], dbar[:], zxyp[:, 0:F])
                nc.vector.tensor_mul(
                    gxy[:, F : 2 * F], dbar[:], zxyp[:, F : 2 * F]
                )
                nc.vector.tensor_add(q[:], sq2[:, 0:F], sq2[:, F : 2 * F])
                nc.vector.tensor_mul(t[:], dbar[:], q[:])
                nc.vector.tensor_mul(dd[:], t[:], h[:])        # = s.q/2
                nc.vector.tensor_mul(ee[:], dbar[:], zlc[:])   # = -d.zl
                lsrc = [(WTd[k + 1], dd), (WTe[k + 1], ee)]

        # ---- output reductions, split so the ux/uy/u tail overlaps the
        # laplacian tail. rp1 rows: 0=ux_A 1=uy_A 3=u_A (+4 chunk B);
        # rp2 rows: 0=S_A 4=S_B ----
        rp2 = ps.tile([8, F], f32, tag="zlp")
        srcs2 = [(1, ee[:]), (0, dd[:])]
        for i, (g, src) in enumerate(srcs2):
            for j in range(NMM):
                s = slice(j * NH, (j + 1) * NH)
                nc.tensor.matmul(
                    rp2[:, s], wlB[:, 8 * g : 8 * g + 8], src[:, s],
                    start=(i == 0), stop=(i == len(srcs2) - 1),
                )
        rp1 = ps.tile([8, F], f32, tag="zp")
        srcs1 = [(2, h[:]), (0, gxy[:, 0:F]), (1, gxy[:, F : 2 * F])]
        for i, (g, src) in enumerate(srcs1):
            for j in range(NMM):
                s = slice(j * NH, (j + 1) * NH)
                nc.tensor.matmul(
                    rp1[:, s], wlA[:, 8 * g : 8 * g + 8], src[:, s],
                    start=(i == 0), stop=(i == len(srcs1) - 1),
                )

        # ---- reshape rows -> per-point [128, 4*FT] tile.
        # T cols: ux 0:16 | uy 16:32 | S 32:48 | u 48:64 ----
        red2 = sb.tile([8, F], f32, tag="red2")
        nc.scalar.copy(red2[:], rp2[:])
        T = sb.tile([128, 4 * FT], f32, tag="T")
        for hh, eng in ((0, nc.sync), (1, nc.gpsimd)):
            eng.dma_start(
                out=T[hh * H : (hh + 1) * H, 2 * FT : 3 * FT],
                in_=red2[4 * hh : 4 * hh + 1, :],
            )
        red1 = sb.tile([8, F], f32, tag="red1")
        nc.scalar.copy(red1[:], rp1[:])
        engs = [nc.sync, nc.scalar, nc.gpsimd]
        for i, (hh, q_) in enumerate(
            (hh, q_) for q_ in (0, 1, 3) for hh in range(2)
        ):
            engs[i % 3].dma_start(
                out=T[hh * H : (hh + 1) * H, q_ * FT : (q_ + 1) * FT],
                in_=red1[4 * hh + q_ : 4 * hh + q_ + 1, :],
            )

        # ---- final assembly (S part first — it lands first now) ----
        m1 = sb.tile([128, 2 * FT], f32, tag="m1")
        p1 = sb.tile([128, FT], f32, tag="p1")
        u_fin = sb.tile([128, FT], f32, tag="u_fin")
        m2f = sb.tile([128, FT], f32, tag="m2f")
        p2 = sb.tile([128, FT], f32, tag="p2")
        pde = sb.tile([128, FT], f32, tag="pde")
        nc.vector.tensor_mul(
            m2f[:], T[:, 2 * FT : 3 * FT], kkft[:, 2 * FT : 3 * FT]
        )
        nc.vector.tensor_add(p2[:], m2f[:], kkft[:, 3 * FT : 4 * FT])
        nc.vector.tensor_mul(m1[:], T[:, 0 : 2 * FT], kkft[:, 0 : 2 * FT])
        nc.vector.tensor_add(p1[:], m1[:, 0:FT], m1[:, FT : 2 * FT])
        nc.vector.tensor_add(pde[:], p1[:], p2[:])
        nc.sync.dma_start(
            out=pde_d[:].rearrange("(p j) -> p j", p=128), in_=pde[:]
        )
        nc.vector.tensor_scalar_add(
            u_fin[:], T[:, 3 * FT : 4 * FT], bwc[:, 7:8]
        )
        nc.scalar.dma_start(
            out=u_d[:].rearrange("(p j) -> p j", p=128), in_=u_fin[:]
        )

    if not nc.is_finalized():
        nc.finalize()
    legalized = _legalize_sync_waits(nc.to_json_bytes())
    nc.to_json_bytes = lambda: legalized
    return nc


_NC = None


def _get_nc():
    global _NC
    if _NC is None:
        # NOTE: walrus LDW-opt stays OFF — it rejects fp16 LDWEIGHTS.
        _NC = build_nc()
    return _NC


def make_in_maps(inputs):
    """Shard full inputs into per-core input maps (host-side packing)."""
    full = {k: np.asarray(v, dtype=np.float32) for k, v in inputs.items()}
    W0 = full["W0"]
    w0r = np.concatenate([W0[:, 0], W0[:, 1]]).astype(np.float16)
    bias_block = np.zeros((8, 128), np.float32)
    for i in range(L):
        bias_block[i] = np.tile(full[f"b{i}"], 2)
    bias_block[4] = np.tile(full["Wout"], 2)
    bias_block[5] = np.tile(W0[:, 0], 2)
    bias_block[6] = np.tile(W0[:, 1], 2)
    bias_block[7] = full["bout"]
    wb = np.concatenate(
        [
            full["W1"].ravel(),
            full["W2"].ravel(),
            full["W3"].ravel(),
            bias_block.ravel(),
        ]
    ).astype(np.float32)

    in_maps = []
    for c in range(NCORES):
        s = slice(c * BC, (c + 1) * BC)
        xy = full["xy"][s]
        kkf = np.concatenate(
            [full["Kx"][s], full["Ky"][s], full["K"][s], full["f"][s]]
        ).astype(np.float32)
        in_maps.append(
            {
                "xc": np.ascontiguousarray(xy[:, 0]).astype(np.float16),
                "yc": np.ascontiguousarray(xy[:, 1]).astype(np.float16),
                "w0r": w0r,
                "wb": wb,
                "kkf": kkf,
            }
        )
    return in_maps


def run(inputs, trace=False, **kw):
    from concourse.bass_utils import run_bass_kernel_spmd

    nc = _get_nc()
    res = run_bass_kernel_spmd(
        nc, make_in_maps(inputs), list(range(NCORES)), trace=trace, **kw
    )
    u = np.concatenate([r["u"] for r in res.results])
    pde = np.concatenate([r["pde"] for r in res.results])
    return (u, pde), res


def kernel(**inputs):
    (u, pde), _ = run(inputs)
    return u, pde
